# revision 9
# baseline (speedup 1.0000x reference)
"""DeepONet Trainium kernel: 8-core SPMD, 3-launch host-synced pipeline.

Math (reference):
  b1 = MLP(x[16,16384]) -> [16,100]; tr = MLP(points[16384,2]) -> [16384,100];
  b2 = MLP(times[100,1]) -> [100,100]; out = einsum('nk,mk,tk->nmt').

Design:
  - Shard M=16384 spatial points across 8 cores (2048 each). Trunk BN needs
    global-M stats; layer-1 stats are computed analytically on-device from
    full-points moments, layers 2/3 stats are host-summed between launches
    (collectives on this runtime cost ~35-90us each; host sync is cheaper).
  - All activations kept transposed [feature(partition), sample(free)].
  - Linear biases before BatchNorm cancel exactly (BN shift invariance) and
    are not computed. BN affine+relu = Relu(z*s + c) with s = gamma*rstd,
    c = beta - mu*s applied per-partition on the scalar engine.
  - einsum: out[n,m,t] = sum_h a3aug[h,m] * A'aug[h,(n,t)] where
    A'aug = [W4|b4]^T-contracted (b1[n]*b2) -- W4/b4 folded in, a3aug has a
    ones row so the bias adds via the same matmul.
"""

import numpy as np

N_CORES = 8
M, NB, T, K, HID, NSP = 16384, 16, 100, 100, 100, 16384
ML = M // N_CORES          # 2048 points per core
KSH = NSP // N_CORES       # 2048 of x-contraction per core
EPS = 1e-5
F32 = np.float32

_CACHE = {}


def _f32(a):
    return np.ascontiguousarray(np.asarray(a, dtype=F32))


# ---------------------------------------------------------------- builders
def _mk(n_extra_setup=None):
    import concourse.bass as bass
    import concourse.tile as tile
    from concourse import bacc, mybir
    nc = bacc.Bacc("TRN2", target_bir_lowering=False, debug=False,
                   num_devices=N_CORES)
    return nc, bass, tile, mybir


def _ap(bass, base_ap, offset, dims):
    return bass.AP(tensor=base_ap.tensor, offset=base_ap.offset + offset,
                   ap=[list(d) for d in dims])


def _bn_finalize(nc, mybir, pool, gsums, gb, n_batch, tagp):
    """gsums [100,2] (sum, sumsq) -> s, c tiles [100,1].
    s = gamma*rstd, c = beta - mu*s.  gb [100,2] = (gamma, beta)."""
    dt = mybir.dt.float32
    mu = pool.tile([100, 1], dt, tag=tagp + "mu")
    e2 = pool.tile([100, 1], dt, tag=tagp + "e2")
    var = pool.tile([100, 1], dt, tag=tagp + "var")
    rstd = pool.tile([100, 1], dt, tag=tagp + "rstd")
    s = pool.tile([100, 1], dt, tag=tagp + "s")
    c = pool.tile([100, 1], dt, tag=tagp + "c")
    inv = 1.0 / float(n_batch)
    nc.vector.tensor_scalar_mul(mu[:], gsums[:, 0:1], inv)
    nc.vector.tensor_scalar_mul(e2[:], gsums[:, 1:2], inv)
    nc.vector.tensor_mul(var[:], mu[:], mu[:])
    nc.vector.tensor_sub(var[:], e2[:], var[:])
    eps_t = pool.tile([100, 1], dt, tag=tagp + "eps")
    nc.vector.memset(eps_t[:], EPS)
    nc.scalar.activation(rstd[:], var[:], mybir.ActivationFunctionType.Sqrt,
                         bias=eps_t[:], scale=1.0)
    nc.vector.reciprocal(rstd[:], rstd[:])
    nc.vector.tensor_mul(s[:], gb[:, 0:1], rstd[:])
    nc.vector.tensor_mul(c[:], mu[:], s[:])
    nc.vector.tensor_sub(c[:], gb[:, 1:2], c[:])
    return s, c


def _local_bn_sc(nc, mybir, pool, z, width, gb, tagp):
    """Local-batch BN over free axis of z [100,width] -> (s, c)."""
    dt = mybir.dt.float32
    st6 = pool.tile([100, 6], dt, tag=tagp + "st6")
    mv = pool.tile([100, 2], dt, tag=tagp + "mv")
    nc.vector.bn_stats(st6[:], z[:, 0:width])
    nc.vector.bn_aggr(mv[:], st6[:])
    sums = pool.tile([100, 2], dt, tag=tagp + "sums")
    nc.vector.tensor_mul(sums[:, 1:2], mv[:, 0:1], mv[:, 0:1])
    nc.vector.tensor_add(sums[:, 1:2], sums[:, 1:2], mv[:, 1:2])
    nc.vector.tensor_scalar_mul(sums[:, 0:1], mv[:, 0:1], float(width))
    nc.vector.tensor_scalar_mul(sums[:, 1:2], sums[:, 1:2], float(width))
    return _bn_finalize(nc, mybir, pool, sums, gb, width, tagp + "f")


def _mlp_tail(nc, mybir, pool, ps_pool, z1, width, ws, b4, gbs, tagp):
    """Branch MLP layers: BN1+relu, L2, BN2+relu, L3, BN3+relu, L4+b4.
    z1 [100,width] in SBUF; ws = [w2T,w3T,w4T] tiles [100,100]; b4 [100,1].
    gbs [100,6] = (g1,b1,g2,b2,g3,b3). Returns out tile [100,width]."""
    dt = mybir.dt.float32
    Relu = mybir.ActivationFunctionType.Relu
    Ident = mybir.ActivationFunctionType.Identity
    cur = z1
    for li in range(3):
        s, c = _local_bn_sc(nc, mybir, pool, cur, width,
                            gbs[:, 2 * li:2 * li + 2], f"{tagp}l{li}")
        a = pool.tile([100, width], dt, tag=f"{tagp}a{li}")
        nc.scalar.activation(a[:], cur[:, 0:width], Relu, bias=c[:], scale=s[:])
        ps = ps_pool.tile([100, width], dt, tag=f"{tagp}ps")
        nc.tensor.matmul(ps[:], ws[li][:], a[:], start=True, stop=True)
        nxt = pool.tile([100, width], dt, tag=f"{tagp}z{li + 1}")
        if li == 2:
            nc.scalar.activation(nxt[:], ps[:], Ident, bias=b4[:], scale=1.0)
        else:
            nc.vector.tensor_copy(nxt[:], ps[:])
        cur = nxt
    return cur


def _build_p1():
    """Launch 1: trunk L1 (analytic BN1) + L2 -> z2, sums2; branch1 z1 partial."""
    nc, bass, tile, mybir = _mk()
    dt = mybir.dt.float32
    Relu = mybir.ActivationFunctionType.Relu

    pT = nc.dram_tensor("pT", [2, ML], dt, kind="ExternalInput").ap()
    pfold = nc.dram_tensor("pfold", [128, 256], dt, kind="ExternalInput").ap()
    xT = nc.dram_tensor("xT", [KSH, 16], dt, kind="ExternalInput").ap()
    w1b1T = nc.dram_tensor("w1b1T", [KSH, 100], dt, kind="ExternalInput").ap()
    w1t_raw = nc.dram_tensor("w1t_raw", [100, 2], dt, kind="ExternalInput").ap()
    w1tT = nc.dram_tensor("w1tT", [2, 100], dt, kind="ExternalInput").ap()
    w2tT = nc.dram_tensor("w2tT", [100, 100], dt, kind="ExternalInput").ap()
    gb1 = nc.dram_tensor("gb1", [100, 2], dt, kind="ExternalInput").ap()

    z2_o = nc.dram_tensor("z2_o", [100, ML], dt, kind="ExternalOutput").ap()
    sums2_o = nc.dram_tensor("sums2_o", [100, 2], dt, kind="ExternalOutput").ap()
    z1b1_o = nc.dram_tensor("z1b1_o", [100, 16], dt, kind="ExternalOutput").ap()

    with tile.TileContext(nc) as tc:
        from contextlib import ExitStack
        with ExitStack() as ctx:
            sb = ctx.enter_context(tc.tile_pool(name="sb", bufs=1))
            sm = ctx.enter_context(tc.tile_pool(name="sm", bufs=1))
            ps = ctx.enter_context(tc.tile_pool(name="ps", bufs=2, space="PSUM"))
            psb = ctx.enter_context(tc.tile_pool(name="psb", bufs=1, space="PSUM"))
            dram = ctx.enter_context(tc.tile_pool(name="dram", bufs=1, space="DRAM"))

            # loads
            pT_sb = sb.tile([2, ML], dt, tag="pT")
            nc.sync.dma_start(pT_sb[:], pT[:])
            pf_sb = sb.tile([128, 128, 2], dt, tag="pf")
            nc.sync.dma_start(pf_sb[:], _ap(bass, pfold, 0,
                              [[256, 128], [2, 128], [1, 2]]))
            xT_sb = sb.tile([128, 16, 16], dt, tag="xT")
            nc.sync.dma_start(xT_sb[:], _ap(bass, xT, 0,
                              [[16, 128], [2048, 16], [1, 16]]))
            wb_sb = sb.tile([128, 16, 100], dt, tag="wb")
            nc.sync.dma_start(wb_sb[:], _ap(bass, w1b1T, 0,
                              [[100, 128], [12800, 16], [1, 100]]))
            w1r_sb = sb.tile([100, 2], dt, tag="w1r")
            nc.sync.dma_start(w1r_sb[:], w1t_raw[:])
            w2t_sb = sb.tile([100, 100], dt, tag="w2t")
            nc.sync.dma_start(w2t_sb[:], w2tT[:])
            gb1_sb = sb.tile([100, 2], dt, tag="gb1")
            nc.sync.dma_start(gb1_sb[:], gb1[:])

            # ---- full-points moments: S0,S1,Q00,Q11,Q01 ----
            p0 = pf_sb[:, :, 0]      # [128,128] stride2
            p1 = pf_sb[:, :, 1]
            prod = sm.tile([128, 128], dt, tag="prod")
            mpart = sm.tile([128, 5], dt, tag="mpart")
            nc.vector.tensor_reduce(mpart[:, 0:1], p0, axis=mybir.AxisListType.X,
                                    op=mybir.AluOpType.add)
            nc.vector.tensor_reduce(mpart[:, 1:2], p1, axis=mybir.AxisListType.X,
                                    op=mybir.AluOpType.add)
            nc.vector.tensor_mul(prod[:], p0, p0)
            nc.vector.tensor_reduce(mpart[:, 2:3], prod[:], axis=mybir.AxisListType.X,
                                    op=mybir.AluOpType.add)
            nc.vector.tensor_mul(prod[:], p1, p1)
            nc.vector.tensor_reduce(mpart[:, 3:4], prod[:], axis=mybir.AxisListType.X,
                                    op=mybir.AluOpType.add)
            nc.vector.tensor_mul(prod[:], p0, p1)
            nc.vector.tensor_reduce(mpart[:, 4:5], prod[:], axis=mybir.AxisListType.X,
                                    op=mybir.AluOpType.add)
            ones = sm.tile([128, 1], dt, tag="ones")
            nc.vector.memset(ones[:], 1.0)
            mps = psb.tile([5, 1], dt, tag="mps")
            nc.tensor.matmul(mps[:], mpart[:], ones[:], start=True, stop=True)
            mom5 = sm.tile([5, 1], dt, tag="mom5")
            nc.vector.tensor_copy(mom5[:], mps[:])
            mom_d = dram.tile([5, 1], dt, tag="mom_d")
            nc.sync.dma_start(mom_d[:], mom5[:])
            momb = sm.tile([100, 5], dt, tag="momb")
            md = mom_d[:, :]
            nc.sync.dma_start(momb[:], _ap(bass, md, 0, [[0, 100], [1, 5]]))

            # analytic stats1: mu1 = W1@Ep ; var1 = w^T C w
            inv = 1.0 / float(M)
            ep = sm.tile([100, 2], dt, tag="ep")
            nc.vector.tensor_scalar_mul(ep[:], momb[:, 0:2], inv)
            q = sm.tile([100, 3], dt, tag="q")   # Q00,Q11,Q01 /M
            nc.vector.tensor_scalar_mul(q[:], momb[:, 2:5], inv)
            cmat = sm.tile([100, 3], dt, tag="cmat")  # C00,C11,C01
            nc.vector.tensor_mul(cmat[:, 0:1], ep[:, 0:1], ep[:, 0:1])
            nc.vector.tensor_mul(cmat[:, 1:2], ep[:, 1:2], ep[:, 1:2])
            nc.vector.tensor_mul(cmat[:, 2:3], ep[:, 0:1], ep[:, 1:2])
            nc.vector.tensor_sub(cmat[:], q[:], cmat[:])
            mu1 = sm.tile([100, 1], dt, tag="mu1")
            t0 = sm.tile([100, 1], dt, tag="t0")
            nc.vector.tensor_mul(mu1[:], w1r_sb[:, 0:1], ep[:, 0:1])
            nc.vector.tensor_mul(t0[:], w1r_sb[:, 1:2], ep[:, 1:2])
            nc.vector.tensor_add(mu1[:], mu1[:], t0[:])
            var1 = sm.tile([100, 1], dt, tag="var1")
            w00 = sm.tile([100, 1], dt, tag="w00")
            nc.vector.tensor_mul(w00[:], w1r_sb[:, 0:1], w1r_sb[:, 0:1])
            nc.vector.tensor_mul(var1[:], w00[:], cmat[:, 0:1])
            nc.vector.tensor_mul(w00[:], w1r_sb[:, 1:2], w1r_sb[:, 1:2])
            nc.vector.tensor_mul(t0[:], w00[:], cmat[:, 1:2])
            nc.vector.tensor_add(var1[:], var1[:], t0[:])
            nc.vector.tensor_mul(w00[:], w1r_sb[:, 0:1], w1r_sb[:, 1:2])
            nc.vector.tensor_mul(t0[:], w00[:], cmat[:, 2:3])
            nc.vector.tensor_scalar_mul(t0[:], t0[:], 2.0)
            nc.vector.tensor_add(var1[:], var1[:], t0[:])
            # s1 = g*rstd ; c1 = b - mu1*s1
            eps_t = sm.tile([100, 1], dt, tag="epst")
            nc.vector.memset(eps_t[:], EPS)
            rstd1 = sm.tile([100, 1], dt, tag="rstd1")
            nc.scalar.activation(rstd1[:], var1[:],
                                 mybir.ActivationFunctionType.Sqrt,
                                 bias=eps_t[:], scale=1.0)
            nc.vector.reciprocal(rstd1[:], rstd1[:])
            s1 = sm.tile([100, 1], dt, tag="s1")
            c1 = sm.tile([100, 1], dt, tag="c1")
            nc.vector.tensor_mul(s1[:], gb1_sb[:, 0:1], rstd1[:])
            nc.vector.tensor_mul(c1[:], mu1[:], s1[:])
            nc.vector.tensor_sub(c1[:], gb1_sb[:, 1:2], c1[:])

            w1tT_sb = sm.tile([2, 100], dt, tag="w1tT")
            nc.sync.dma_start(w1tT_sb[:], w1tT[:])

            # ---- trunk L1 + affine1 -> a1 ; L2 -> z2, stats ----
            a1 = sb.tile([100, ML], dt, tag="a1")
            for ci in range(4):
                zp = ps.tile([100, 512], dt, tag="zp")
                nc.tensor.matmul(zp[:], w1tT_sb[:], pT_sb[:, ci * 512:(ci + 1) * 512],
                                 start=True, stop=True)
                nc.scalar.activation(a1[:, ci * 512:(ci + 1) * 512], zp[:],
                                     Relu, bias=c1[:], scale=s1[:])
            z2 = sb.tile([100, ML], dt, tag="z2")
            st6 = sm.tile([100, 4, 6], dt, tag="st6z2")
            for ci in range(4):
                zp = ps.tile([100, 512], dt, tag="zp")
                nc.tensor.matmul(zp[:], w2t_sb[:], a1[:, ci * 512:(ci + 1) * 512],
                                 start=True, stop=True)
                nc.vector.bn_stats(st6[:, ci, :], zp[:])
                nc.scalar.copy(z2[:, ci * 512:(ci + 1) * 512], zp[:])
            mv = sm.tile([100, 2], dt, tag="mvz2")
            nc.vector.bn_aggr(mv[:], st6[:])
            sums2 = sm.tile([100, 2], dt, tag="sums2")
            nc.vector.tensor_mul(sums2[:, 1:2], mv[:, 0:1], mv[:, 0:1])
            nc.vector.tensor_add(sums2[:, 1:2], sums2[:, 1:2], mv[:, 1:2])
            nc.vector.tensor_scalar_mul(sums2[:, 0:1], mv[:, 0:1], float(ML))
            nc.vector.tensor_scalar_mul(sums2[:, 1:2], sums2[:, 1:2], float(ML))

            # ---- branch1 z1 partial: accumulate 16 k-chunks ----
            zb = psb.tile([100, 16], dt, tag="zb")
            for kt in range(16):
                nc.tensor.matmul(zb[:], wb_sb[:, kt, :], xT_sb[:, kt, :],
                                 start=(kt == 0), stop=(kt == 15))
            z1b1 = sm.tile([100, 16], dt, tag="z1b1")
            nc.vector.tensor_copy(z1b1[:], zb[:])

            # outputs
            nc.sync.dma_start(z2_o[:], z2[:])
            nc.sync.dma_start(sums2_o[:], sums2[:])
            nc.sync.dma_start(z1b1_o[:], z1b1[:])
    nc.compile()
    return nc


def _build_p2():
    """Launch 2: affine2 -> a2 -> L3 -> z3,sums3 ; branch1 tail; branch2 full."""
    nc, bass, tile, mybir = _mk()
    dt = mybir.dt.float32
    Relu = mybir.ActivationFunctionType.Relu

    z2_i = nc.dram_tensor("z2_i", [100, ML], dt, kind="ExternalInput").ap()
    gs2 = nc.dram_tensor("gs2", [100, 2], dt, kind="ExternalInput").ap()
    z1g = nc.dram_tensor("z1g", [100, 16], dt, kind="ExternalInput").ap()
    tT = nc.dram_tensor("tT", [1, 100], dt, kind="ExternalInput").ap()
    w3tT = nc.dram_tensor("w3tT", [100, 100], dt, kind="ExternalInput").ap()
    wb1 = nc.dram_tensor("wb1", [100, 300], dt, kind="ExternalInput").ap()  # w2,w3,w4 b1T
    b4b1 = nc.dram_tensor("b4b1", [100, 1], dt, kind="ExternalInput").ap()
    w1b2T = nc.dram_tensor("w1b2T", [1, 100], dt, kind="ExternalInput").ap()
    wb2 = nc.dram_tensor("wb2", [100, 300], dt, kind="ExternalInput").ap()
    b4b2 = nc.dram_tensor("b4b2", [100, 1], dt, kind="ExternalInput").ap()
    gbs = nc.dram_tensor("gbs", [100, 14], dt, kind="ExternalInput").ap()
    # gbs cols: 0:2 trunk-BN2, 2:8 b1-BN1..3, 8:14 b2-BN1..3

    z3_o = nc.dram_tensor("z3_o", [100, ML], dt, kind="ExternalOutput").ap()
    sums3_o = nc.dram_tensor("sums3_o", [100, 2], dt, kind="ExternalOutput").ap()
    b1T_o = nc.dram_tensor("b1T_o", [100, 16], dt, kind="ExternalOutput").ap()
    b2T_o = nc.dram_tensor("b2T_o", [100, 100], dt, kind="ExternalOutput").ap()

    with tile.TileContext(nc) as tc:
        from contextlib import ExitStack
        with ExitStack() as ctx:
            sb = ctx.enter_context(tc.tile_pool(name="sb", bufs=1))
            sm = ctx.enter_context(tc.tile_pool(name="sm", bufs=1))
            ps = ctx.enter_context(tc.tile_pool(name="ps", bufs=2, space="PSUM"))
            psb = ctx.enter_context(tc.tile_pool(name="psb", bufs=2, space="PSUM"))

            z2_sb = sb.tile([100, ML], dt, tag="z2")
            nc.sync.dma_start(z2_sb[:], z2_i[:])
            gs2_sb = sm.tile([100, 2], dt, tag="gs2")
            nc.sync.dma_start(gs2_sb[:], gs2[:])
            z1g_sb = sm.tile([100, 16], dt, tag="z1g")
            nc.sync.dma_start(z1g_sb[:], z1g[:])
            tT_sb = sm.tile([1, 100], dt, tag="tT")
            nc.sync.dma_start(tT_sb[:], tT[:])
            w3t_sb = sb.tile([100, 100], dt, tag="w3t")
            nc.sync.dma_start(w3t_sb[:], w3tT[:])
            wb1_sb = sb.tile([100, 300], dt, tag="wb1")
            nc.sync.dma_start(wb1_sb[:], wb1[:])
            b4b1_sb = sm.tile([100, 1], dt, tag="b4b1")
            nc.sync.dma_start(b4b1_sb[:], b4b1[:])
            w1b2_sb = sm.tile([1, 100], dt, tag="w1b2")
            nc.sync.dma_start(w1b2_sb[:], w1b2T[:])
            wb2_sb = sb.tile([100, 300], dt, tag="wb2")
            nc.sync.dma_start(wb2_sb[:], wb2[:])
            b4b2_sb = sm.tile([100, 1], dt, tag="b4b2")
            nc.sync.dma_start(b4b2_sb[:], b4b2[:])
            gbs_sb = sm.tile([100, 14], dt, tag="gbs")
            nc.sync.dma_start(gbs_sb[:], gbs[:])

            # trunk: affine2 -> a2 -> L3 -> z3, stats
            s2, c2 = _bn_finalize(nc, mybir, sm, gs2_sb, gbs_sb[:, 0:2], M, "t2")
            a2 = sb.tile([100, ML], dt, tag="a2")
            nc.scalar.activation(a2[:], z2_sb[:], Relu, bias=c2[:], scale=s2[:])
            z3 = sb.tile([100, ML], dt, tag="z3")
            st6 = sm.tile([100, 4, 6], dt, tag="st6z3")
            for ci in range(4):
                zp = ps.tile([100, 512], dt, tag="zp")
                nc.tensor.matmul(zp[:], w3t_sb[:], a2[:, ci * 512:(ci + 1) * 512],
                                 start=True, stop=True)
                nc.vector.bn_stats(st6[:, ci, :], zp[:])
                nc.scalar.copy(z3[:, ci * 512:(ci + 1) * 512], zp[:])
            mv = sm.tile([100, 2], dt, tag="mvz3")
            nc.vector.bn_aggr(mv[:], st6[:])
            sums3 = sm.tile([100, 2], dt, tag="sums3")
            nc.vector.tensor_mul(sums3[:, 1:2], mv[:, 0:1], mv[:, 0:1])
            nc.vector.tensor_add(sums3[:, 1:2], sums3[:, 1:2], mv[:, 1:2])
            nc.vector.tensor_scalar_mul(sums3[:, 0:1], mv[:, 0:1], float(ML))
            nc.vector.tensor_scalar_mul(sums3[:, 1:2], sums3[:, 1:2], float(ML))

            # branch1 tail
            wsb1 = [wb1_sb[:, 0:100], wb1_sb[:, 100:200], wb1_sb[:, 200:300]]
            b1T = _mlp_tail(nc, mybir, sm, psb, z1g_sb, 16, wsb1, b4b1_sb,
                            gbs_sb[:, 2:8], "b1")
            # branch2 full
            zb2 = psb.tile([100, 100], dt, tag="zb2")
            nc.tensor.matmul(zb2[:], w1b2_sb[:], tT_sb[:], start=True, stop=True)
            z1b2 = sm.tile([100, 100], dt, tag="z1b2")
            nc.vector.tensor_copy(z1b2[:], zb2[:])
            wsb2 = [wb2_sb[:, 0:100], wb2_sb[:, 100:200], wb2_sb[:, 200:300]]
            b2T = _mlp_tail(nc, mybir, sm, psb, z1b2, 100, wsb2, b4b2_sb,
                            gbs_sb[:, 8:14], "b2")

            nc.sync.dma_start(z3_o[:], z3[:])
            nc.sync.dma_start(sums3_o[:], sums3[:])
            nc.sync.dma_start(b1T_o[:], b1T[:])
            nc.sync.dma_start(b2T_o[:], b2T[:])
    nc.compile()
    return nc


def _build_p3():
    """Launch 3: affine3 -> a3aug ; A'aug = [W4|b4]^T AT ; contraction; out."""
    nc, bass, tile, mybir = _mk()
    dt = mybir.dt.float32
    Relu = mybir.ActivationFunctionType.Relu

    z3_i = nc.dram_tensor("z3_i", [100, ML], dt, kind="ExternalInput").ap()
    gs3 = nc.dram_tensor("gs3", [100, 2], dt, kind="ExternalInput").ap()
    b1T_i = nc.dram_tensor("b1T_i", [100, 16], dt, kind="ExternalInput").ap()
    b2T_i = nc.dram_tensor("b2T_i", [100, 100], dt, kind="ExternalInput").ap()
    w4tT = nc.dram_tensor("w4tT", [100, 100], dt, kind="ExternalInput").ap()
    b4t = nc.dram_tensor("b4t", [100, 1], dt, kind="ExternalInput").ap()
    gb3 = nc.dram_tensor("gb3", [100, 2], dt, kind="ExternalInput").ap()
    out = nc.dram_tensor("out", [NB, ML, T], dt, kind="ExternalOutput").ap()

    with tile.TileContext(nc) as tc:
        from contextlib import ExitStack
        with ExitStack() as ctx:
            sb = ctx.enter_context(tc.tile_pool(name="sb", bufs=1))
            sm = ctx.enter_context(tc.tile_pool(name="sm", bufs=1))
            osb = ctx.enter_context(tc.tile_pool(name="osb", bufs=3))
            ps = ctx.enter_context(tc.tile_pool(name="ps", bufs=2, space="PSUM"))
            psc = ctx.enter_context(tc.tile_pool(name="psc", bufs=2, space="PSUM"))

            z3_sb = sb.tile([100, ML], dt, tag="z3")
            nc.sync.dma_start(z3_sb[:], z3_i[:])
            gs3_sb = sm.tile([100, 2], dt, tag="gs3")
            nc.sync.dma_start(gs3_sb[:], gs3[:])
            b1T_sb = sm.tile([100, 16], dt, tag="b1T")
            nc.sync.dma_start(b1T_sb[:], b1T_i[:])
            b2T_sb = sm.tile([100, 100], dt, tag="b2T")
            nc.sync.dma_start(b2T_sb[:], b2T_i[:])
            w4_sb = sb.tile([100, 100], dt, tag="w4")
            nc.sync.dma_start(w4_sb[:], w4tT[:])
            b4_sb = sm.tile([100, 1], dt, tag="b4")
            nc.sync.dma_start(b4_sb[:], b4t[:])
            gb3_sb = sm.tile([100, 2], dt, tag="gb3")
            nc.sync.dma_start(gb3_sb[:], gb3[:])

            Ident = mybir.ActivationFunctionType.Identity
            # affine3 -> a3 ; L4 (+b4) -> trT
            s3, c3 = _bn_finalize(nc, mybir, sm, gs3_sb, gb3_sb, M, "t3")
            a3 = sb.tile([100, ML], dt, tag="a3")
            nc.scalar.activation(a3[:], z3_sb[:], Relu, bias=c3[:], scale=s3[:])
            trT = sb.tile([100, ML], dt, tag="trT")
            for ci in range(4):
                zp = ps.tile([100, 512], dt, tag="zp")
                nc.tensor.matmul(zp[:], w4_sb[:], a3[:, ci * 512:(ci + 1) * 512],
                                 start=True, stop=True)
                nc.scalar.activation(trT[:, ci * 512:(ci + 1) * 512], zp[:],
                                     Ident, bias=b4_sb[:], scale=1.0)

            # AT [100, 1600]: AT[:, n*100:(n+1)*100] = b2T * b1T[:, n]
            AT = sb.tile([100, 1600], dt, tag="AT")
            for n in range(NB):
                nc.vector.tensor_scalar_mul(AT[:, n * 100:(n + 1) * 100],
                                            b2T_sb[:], b1T_sb[:, n:n + 1])

            # contraction: per m-tile (16 of 128), two halves of 800 cols
            for mt in range(16):
                ot = osb.tile([128, 1600], dt, tag="ot")
                for hf in range(2):
                    cp = psc.tile([128, 1024], dt, tag="cp")
                    base = hf * 800
                    nc.tensor.matmul(cp[:, 0:512],
                                     trT[:, mt * 128:(mt + 1) * 128],
                                     AT[:, base:base + 512],
                                     start=True, stop=True)
                    nc.tensor.matmul(cp[:, 512:800],
                                     trT[:, mt * 128:(mt + 1) * 128],
                                     AT[:, base + 512:base + 800],
                                     start=True, stop=True)
                    nc.scalar.copy(ot[:, base:base + 400], cp[:, 0:400])
                    nc.vector.tensor_copy(ot[:, base + 400:base + 800],
                                          cp[:, 400:800])
                    for n in range(hf * 8, hf * 8 + 8):
                        dst = _ap(bass, out, n * ML * T + mt * 128 * T,
                                  [[T, 128], [1, T]])
                        nc.sync.dma_start(dst,
                                          ot[:, n * 100:(n + 1) * 100])
    nc.compile()
    return nc


def _get_programs():
    if "p1" not in _CACHE:
        _CACHE["p1"] = _build_p1()
        _CACHE["p2"] = _build_p2()
        _CACHE["p3"] = _build_p3()
    return _CACHE["p1"], _CACHE["p2"], _CACHE["p3"]


def _run(nc, in_maps, **kw):
    from concourse import bass_utils
    return bass_utils.run_bass_kernel_spmd(nc, in_maps,
                                           core_ids=list(range(N_CORES)), **kw)


def kernel(x, points, times, branch1_params, branch2_params, trunk_params,
           _timings=None):
    x = _f32(x); points = _f32(points); times = _f32(times)
    b1p = [tuple(_f32(t) for t in tup) for tup in branch1_params]
    b2p = [tuple(_f32(t) for t in tup) for tup in branch2_params]
    trp = [tuple(_f32(t) for t in tup) for tup in trunk_params]
    # params: [(W1,b1),(g1,be1),(W2,b2),(g2,be2),(W3,b3),(g3,be3),(W4,b4)]
    p1, p2, p3 = _get_programs()

    pT_full = np.ascontiguousarray(points.T)              # [2, 16384]
    pfold = np.ascontiguousarray(points.reshape(128, 256))
    xT_full = np.ascontiguousarray(x.T)                   # [16384, 16]
    w1b1T_full = np.ascontiguousarray(b1p[0][0].T)        # [16384, 100]

    def gb(p, i):  # (gamma, beta) of BN i (0,1,2) as [100,2]
        g, be = p[2 * i + 1]
        return np.ascontiguousarray(np.stack([g, be], axis=1))

    in1 = []
    for c in range(N_CORES):
        in1.append({
            "pT": np.ascontiguousarray(pT_full[:, c * ML:(c + 1) * ML]),
            "pfold": pfold,
            "xT": np.ascontiguousarray(xT_full[c * KSH:(c + 1) * KSH]),
            "w1b1T": np.ascontiguousarray(w1b1T_full[c * KSH:(c + 1) * KSH]),
            "w1t_raw": trp[0][0],                          # [100,2]
            "w1tT": np.ascontiguousarray(trp[0][0].T),     # [2,100]
            "w2tT": np.ascontiguousarray(trp[2][0].T),     # [100,100]
            "gb1": gb(trp, 0),
        })
    r1 = _run(p1, in1, trace=bool(_timings))
    if _timings is not None:
        _timings.append(r1.exec_time_ns)

    z1g = np.sum([r1.results[c]["z1b1_o"].astype(np.float64)
                  for c in range(N_CORES)], axis=0).astype(F32)
    gs2 = np.sum([r1.results[c]["sums2_o"].astype(np.float64)
                  for c in range(N_CORES)], axis=0).astype(F32)

    def wpack(p):
        return np.ascontiguousarray(np.concatenate(
            [p[2][0].T, p[4][0].T, p[6][0].T], axis=1))    # [100, 300]

    gbs = np.concatenate([gb(trp, 1), gb(b1p, 0), gb(b1p, 1), gb(b1p, 2),
                          gb(b2p, 0), gb(b2p, 1), gb(b2p, 2)], axis=1)
    in2 = []
    for c in range(N_CORES):
        in2.append({
            "z2_i": r1.results[c]["z2_o"],
            "gs2": gs2, "z1g": z1g,
            "tT": np.ascontiguousarray(times.T),           # [1,100]
            "w3tT": np.ascontiguousarray(trp[4][0].T),
            "wb1": wpack(b1p), "b4b1": b1p[6][1].reshape(100, 1),
            "w1b2T": np.ascontiguousarray(b2p[0][0].T),    # [1,100]
            "wb2": wpack(b2p), "b4b2": b2p[6][1].reshape(100, 1),
            "gbs": np.ascontiguousarray(gbs),
        })
    r2 = _run(p2, in2, trace=bool(_timings))
    if _timings is not None:
        _timings.append(r2.exec_time_ns)

    gs3 = np.sum([r2.results[c]["sums3_o"].astype(np.float64)
                  for c in range(N_CORES)], axis=0).astype(F32)
    in3 = []
    for c in range(N_CORES):
        in3.append({
            "z3_i": r2.results[c]["z3_o"],
            "gs3": gs3,
            "b1T_i": r2.results[c]["b1T_o"],
            "b2T_i": r2.results[c]["b2T_o"],
            "w4tT": np.ascontiguousarray(trp[6][0].T),
            "b4t": trp[6][1].reshape(100, 1),
            "gb3": gb(trp, 2),
        })
    r3 = _run(p3, in3, trace=bool(_timings))
    if _timings is not None:
        _timings.append(r3.exec_time_ns)

    return np.concatenate([r3.results[c]["out"] for c in range(N_CORES)],
                          axis=1)


# revision 10
# speedup vs baseline: 1.5944x; 1.5944x over previous
"""DeepONet Trainium kernel: 8-core SPMD, 3-launch host-synced pipeline.

Math (reference):
  b1 = MLP(x[16,16384]) -> [16,100]; tr = MLP(points[16384,2]) -> [16384,100];
  b2 = MLP(times[100,1]) -> [100,100]; out = einsum('nk,mk,tk->nmt').

Design:
  - Shard M=16384 spatial points across 8 cores (2048 each). Trunk BN needs
    global-M stats; layer-1 stats are computed analytically on-device from
    full-points moments, layers 2/3 stats are host-summed between launches
    (collectives on this runtime cost ~35-90us each; host sync is cheaper).
  - All activations kept transposed [feature(partition), sample(free)].
  - Linear biases before BatchNorm cancel exactly (BN shift invariance) and
    are not computed. BN affine+relu = Relu(z*s + c) with s = gamma*rstd,
    c = beta - mu*s applied per-partition on the scalar engine.
  - einsum: out[n,m,t] = sum_h a3aug[h,m] * A'aug[h,(n,t)] where
    A'aug = [W4|b4]^T-contracted (b1[n]*b2) -- W4/b4 folded in, a3aug has a
    ones row so the bias adds via the same matmul.
"""

import numpy as np

N_CORES = 8
M, NB, T, K, HID, NSP = 16384, 16, 100, 100, 100, 16384
ML = M // N_CORES          # 2048 points per core
KSH = NSP // N_CORES       # 2048 of x-contraction per core
EPS = 1e-5
F32 = np.float32

_CACHE = {}


def _f32(a):
    return np.ascontiguousarray(np.asarray(a, dtype=F32))


# ---------------------------------------------------------------- builders
def _mk(n_extra_setup=None):
    import concourse.bass as bass
    import concourse.tile as tile
    from concourse import bacc, mybir
    nc = bacc.Bacc("TRN2", target_bir_lowering=False, debug=False,
                   num_devices=N_CORES)
    return nc, bass, tile, mybir


def _ap(bass, base_ap, offset, dims):
    return bass.AP(tensor=base_ap.tensor, offset=base_ap.offset + offset,
                   ap=[list(d) for d in dims])


def _bn_finalize(nc, mybir, pool, gsums, gb, n_batch, tagp):
    """gsums [100,2] (sum, sumsq) -> s, c tiles [100,1].
    s = gamma*rstd, c = beta - mu*s.  gb [100,2] = (gamma, beta)."""
    dt = mybir.dt.float32
    mu = pool.tile([100, 1], dt, tag=tagp + "mu")
    e2 = pool.tile([100, 1], dt, tag=tagp + "e2")
    var = pool.tile([100, 1], dt, tag=tagp + "var")
    rstd = pool.tile([100, 1], dt, tag=tagp + "rstd")
    s = pool.tile([100, 1], dt, tag=tagp + "s")
    c = pool.tile([100, 1], dt, tag=tagp + "c")
    inv = 1.0 / float(n_batch)
    nc.vector.tensor_scalar_mul(mu[:], gsums[:, 0:1], inv)
    nc.vector.tensor_scalar_mul(e2[:], gsums[:, 1:2], inv)
    nc.vector.tensor_mul(var[:], mu[:], mu[:])
    nc.vector.tensor_sub(var[:], e2[:], var[:])
    eps_t = pool.tile([100, 1], dt, tag=tagp + "eps")
    nc.vector.memset(eps_t[:], EPS)
    nc.scalar.activation(rstd[:], var[:], mybir.ActivationFunctionType.Sqrt,
                         bias=eps_t[:], scale=1.0)
    nc.vector.reciprocal(rstd[:], rstd[:])
    nc.vector.tensor_mul(s[:], gb[:, 0:1], rstd[:])
    nc.vector.tensor_mul(c[:], mu[:], s[:])
    nc.vector.tensor_sub(c[:], gb[:, 1:2], c[:])
    return s, c


def _local_bn_sc(nc, mybir, pool, z, width, gb, tagp):
    """Local-batch BN over free axis of z [100,width] -> (s, c)."""
    dt = mybir.dt.float32
    st6 = pool.tile([100, 6], dt, tag=tagp + "st6")
    mv = pool.tile([100, 2], dt, tag=tagp + "mv")
    nc.vector.bn_stats(st6[:], z[:, 0:width])
    nc.vector.bn_aggr(mv[:], st6[:])
    sums = pool.tile([100, 2], dt, tag=tagp + "sums")
    nc.vector.tensor_mul(sums[:, 1:2], mv[:, 0:1], mv[:, 0:1])
    nc.vector.tensor_add(sums[:, 1:2], sums[:, 1:2], mv[:, 1:2])
    nc.vector.tensor_scalar_mul(sums[:, 0:1], mv[:, 0:1], float(width))
    nc.vector.tensor_scalar_mul(sums[:, 1:2], sums[:, 1:2], float(width))
    return _bn_finalize(nc, mybir, pool, sums, gb, width, tagp + "f")


def _mlp_tail(nc, mybir, pool, ps_pool, z1, width, ws, b4, gbs, tagp):
    """Branch MLP layers: BN1+relu, L2, BN2+relu, L3, BN3+relu, L4+b4.
    z1 [100,width] in SBUF; ws = [w2T,w3T,w4T] tiles [100,100]; b4 [100,1].
    gbs [100,6] = (g1,b1,g2,b2,g3,b3). Returns out tile [100,width]."""
    dt = mybir.dt.float32
    Relu = mybir.ActivationFunctionType.Relu
    Ident = mybir.ActivationFunctionType.Identity
    cur = z1
    for li in range(3):
        s, c = _local_bn_sc(nc, mybir, pool, cur, width,
                            gbs[:, 2 * li:2 * li + 2], f"{tagp}l{li}")
        a = pool.tile([100, width], dt, tag=f"{tagp}a{li}")
        nc.scalar.activation(a[:], cur[:, 0:width], Relu, bias=c[:], scale=s[:])
        ps = ps_pool.tile([100, width], dt, tag=f"{tagp}ps")
        nc.tensor.matmul(ps[:], ws[li][:], a[:], start=True, stop=True)
        nxt = pool.tile([100, width], dt, tag=f"{tagp}z{li + 1}")
        if li == 2:
            nc.scalar.activation(nxt[:], ps[:], Ident, bias=b4[:], scale=1.0)
        else:
            nc.vector.tensor_copy(nxt[:], ps[:])
        cur = nxt
    return cur


def _build_p1():
    """Launch 1: trunk L1 (analytic BN1) + L2 -> z2, sums2; branch1 z1 partial."""
    nc, bass, tile, mybir = _mk()
    dt = mybir.dt.float32
    Relu = mybir.ActivationFunctionType.Relu

    pT = nc.dram_tensor("pT", [2, ML], dt, kind="ExternalInput").ap()
    pfold = nc.dram_tensor("pfold", [128, 256], dt, kind="ExternalInput").ap()
    xT = nc.dram_tensor("xT", [KSH, 16], dt, kind="ExternalInput").ap()
    w1b1T = nc.dram_tensor("w1b1T", [KSH, 100], dt, kind="ExternalInput").ap()
    w1t_raw = nc.dram_tensor("w1t_raw", [100, 2], dt, kind="ExternalInput").ap()
    w1tT = nc.dram_tensor("w1tT", [2, 100], dt, kind="ExternalInput").ap()
    w2tT = nc.dram_tensor("w2tT", [100, 100], dt, kind="ExternalInput").ap()
    gb1 = nc.dram_tensor("gb1", [100, 2], dt, kind="ExternalInput").ap()

    z2_o = nc.dram_tensor("z2_o", [100, ML], dt, kind="ExternalOutput").ap()
    sums2_o = nc.dram_tensor("sums2_o", [100, 2], dt, kind="ExternalOutput").ap()
    z1b1_o = nc.dram_tensor("z1b1_o", [100, 16], dt, kind="ExternalOutput").ap()

    with tile.TileContext(nc) as tc:
        from contextlib import ExitStack
        with ExitStack() as ctx:
            sb = ctx.enter_context(tc.tile_pool(name="sb", bufs=1))
            sm = ctx.enter_context(tc.tile_pool(name="sm", bufs=1))
            ps = ctx.enter_context(tc.tile_pool(name="ps", bufs=2, space="PSUM"))
            psb = ctx.enter_context(tc.tile_pool(name="psb", bufs=1, space="PSUM"))
            dram = ctx.enter_context(tc.tile_pool(name="dram", bufs=1, space="DRAM"))

            # loads
            pT_sb = sb.tile([2, ML], dt, tag="pT")
            nc.sync.dma_start(pT_sb[:], pT[:])
            pf_sb = sb.tile([128, 128, 2], dt, tag="pf")
            nc.sync.dma_start(pf_sb[:], _ap(bass, pfold, 0,
                              [[256, 128], [2, 128], [1, 2]]))
            xT_sb = sb.tile([128, 16, 16], dt, tag="xT")
            nc.sync.dma_start(xT_sb[:], _ap(bass, xT, 0,
                              [[16, 128], [2048, 16], [1, 16]]))
            wb_sb = sb.tile([128, 16, 100], dt, tag="wb")
            nc.sync.dma_start(wb_sb[:], _ap(bass, w1b1T, 0,
                              [[100, 128], [12800, 16], [1, 100]]))
            w1r_sb = sb.tile([100, 2], dt, tag="w1r")
            nc.sync.dma_start(w1r_sb[:], w1t_raw[:])
            w2t_sb = sb.tile([100, 100], dt, tag="w2t")
            nc.sync.dma_start(w2t_sb[:], w2tT[:])
            gb1_sb = sb.tile([100, 2], dt, tag="gb1")
            nc.sync.dma_start(gb1_sb[:], gb1[:])

            # ---- full-points moments: S0,S1,Q00,Q11,Q01 ----
            p0 = pf_sb[:, :, 0]      # [128,128] stride2
            p1 = pf_sb[:, :, 1]
            prod = sm.tile([128, 128], dt, tag="prod")
            mpart = sm.tile([128, 5], dt, tag="mpart")
            nc.vector.tensor_reduce(mpart[:, 0:1], p0, axis=mybir.AxisListType.X,
                                    op=mybir.AluOpType.add)
            nc.vector.tensor_reduce(mpart[:, 1:2], p1, axis=mybir.AxisListType.X,
                                    op=mybir.AluOpType.add)
            nc.vector.tensor_mul(prod[:], p0, p0)
            nc.vector.tensor_reduce(mpart[:, 2:3], prod[:], axis=mybir.AxisListType.X,
                                    op=mybir.AluOpType.add)
            nc.vector.tensor_mul(prod[:], p1, p1)
            nc.vector.tensor_reduce(mpart[:, 3:4], prod[:], axis=mybir.AxisListType.X,
                                    op=mybir.AluOpType.add)
            nc.vector.tensor_mul(prod[:], p0, p1)
            nc.vector.tensor_reduce(mpart[:, 4:5], prod[:], axis=mybir.AxisListType.X,
                                    op=mybir.AluOpType.add)
            ones = sm.tile([128, 1], dt, tag="ones")
            nc.vector.memset(ones[:], 1.0)
            mps = psb.tile([5, 1], dt, tag="mps")
            nc.tensor.matmul(mps[:], mpart[:], ones[:], start=True, stop=True)
            mom5 = sm.tile([5, 1], dt, tag="mom5")
            nc.vector.tensor_copy(mom5[:], mps[:])
            mom_d = dram.tile([5, 1], dt, tag="mom_d")
            nc.sync.dma_start(mom_d[:], mom5[:])
            momb = sm.tile([100, 5], dt, tag="momb")
            md = mom_d[:, :]
            nc.sync.dma_start(momb[:], _ap(bass, md, 0, [[0, 100], [1, 5]]))

            # analytic stats1: mu1 = W1@Ep ; var1 = w^T C w
            inv = 1.0 / float(M)
            ep = sm.tile([100, 2], dt, tag="ep")
            nc.vector.tensor_scalar_mul(ep[:], momb[:, 0:2], inv)
            q = sm.tile([100, 3], dt, tag="q")   # Q00,Q11,Q01 /M
            nc.vector.tensor_scalar_mul(q[:], momb[:, 2:5], inv)
            cmat = sm.tile([100, 3], dt, tag="cmat")  # C00,C11,C01
            nc.vector.tensor_mul(cmat[:, 0:1], ep[:, 0:1], ep[:, 0:1])
            nc.vector.tensor_mul(cmat[:, 1:2], ep[:, 1:2], ep[:, 1:2])
            nc.vector.tensor_mul(cmat[:, 2:3], ep[:, 0:1], ep[:, 1:2])
            nc.vector.tensor_sub(cmat[:], q[:], cmat[:])
            mu1 = sm.tile([100, 1], dt, tag="mu1")
            t0 = sm.tile([100, 1], dt, tag="t0")
            nc.vector.tensor_mul(mu1[:], w1r_sb[:, 0:1], ep[:, 0:1])
            nc.vector.tensor_mul(t0[:], w1r_sb[:, 1:2], ep[:, 1:2])
            nc.vector.tensor_add(mu1[:], mu1[:], t0[:])
            var1 = sm.tile([100, 1], dt, tag="var1")
            w00 = sm.tile([100, 1], dt, tag="w00")
            nc.vector.tensor_mul(w00[:], w1r_sb[:, 0:1], w1r_sb[:, 0:1])
            nc.vector.tensor_mul(var1[:], w00[:], cmat[:, 0:1])
            nc.vector.tensor_mul(w00[:], w1r_sb[:, 1:2], w1r_sb[:, 1:2])
            nc.vector.tensor_mul(t0[:], w00[:], cmat[:, 1:2])
            nc.vector.tensor_add(var1[:], var1[:], t0[:])
            nc.vector.tensor_mul(w00[:], w1r_sb[:, 0:1], w1r_sb[:, 1:2])
            nc.vector.tensor_mul(t0[:], w00[:], cmat[:, 2:3])
            nc.vector.tensor_scalar_mul(t0[:], t0[:], 2.0)
            nc.vector.tensor_add(var1[:], var1[:], t0[:])
            # s1 = g*rstd ; c1 = b - mu1*s1
            eps_t = sm.tile([100, 1], dt, tag="epst")
            nc.vector.memset(eps_t[:], EPS)
            rstd1 = sm.tile([100, 1], dt, tag="rstd1")
            nc.scalar.activation(rstd1[:], var1[:],
                                 mybir.ActivationFunctionType.Sqrt,
                                 bias=eps_t[:], scale=1.0)
            nc.vector.reciprocal(rstd1[:], rstd1[:])
            s1 = sm.tile([100, 1], dt, tag="s1")
            c1 = sm.tile([100, 1], dt, tag="c1")
            nc.vector.tensor_mul(s1[:], gb1_sb[:, 0:1], rstd1[:])
            nc.vector.tensor_mul(c1[:], mu1[:], s1[:])
            nc.vector.tensor_sub(c1[:], gb1_sb[:, 1:2], c1[:])

            w1tT_sb = sm.tile([2, 100], dt, tag="w1tT")
            nc.sync.dma_start(w1tT_sb[:], w1tT[:])

            # ---- trunk L1 + affine1 -> a1 ; L2 -> z2, stats ----
            a1 = sb.tile([100, ML], dt, tag="a1")
            for ci in range(4):
                zp = ps.tile([100, 512], dt, tag="zp")
                nc.tensor.matmul(zp[:], w1tT_sb[:], pT_sb[:, ci * 512:(ci + 1) * 512],
                                 start=True, stop=True)
                nc.scalar.activation(a1[:, ci * 512:(ci + 1) * 512], zp[:],
                                     Relu, bias=c1[:], scale=s1[:])
            z2 = sb.tile([100, ML], dt, tag="z2")
            st6 = sm.tile([100, 4, 6], dt, tag="st6z2")
            for ci in range(4):
                zp = ps.tile([100, 512], dt, tag="zp")
                nc.tensor.matmul(zp[:], w2t_sb[:], a1[:, ci * 512:(ci + 1) * 512],
                                 start=True, stop=True)
                nc.vector.bn_stats(st6[:, ci, :], zp[:])
                nc.scalar.copy(z2[:, ci * 512:(ci + 1) * 512], zp[:])
            mv = sm.tile([100, 2], dt, tag="mvz2")
            nc.vector.bn_aggr(mv[:], st6[:])
            sums2 = sm.tile([100, 2], dt, tag="sums2")
            nc.vector.tensor_mul(sums2[:, 1:2], mv[:, 0:1], mv[:, 0:1])
            nc.vector.tensor_add(sums2[:, 1:2], sums2[:, 1:2], mv[:, 1:2])
            nc.vector.tensor_scalar_mul(sums2[:, 0:1], mv[:, 0:1], float(ML))
            nc.vector.tensor_scalar_mul(sums2[:, 1:2], sums2[:, 1:2], float(ML))

            # ---- branch1 z1 partial: accumulate 16 k-chunks ----
            zb = psb.tile([100, 16], dt, tag="zb")
            for kt in range(16):
                nc.tensor.matmul(zb[:], wb_sb[:, kt, :], xT_sb[:, kt, :],
                                 start=(kt == 0), stop=(kt == 15))
            z1b1 = sm.tile([100, 16], dt, tag="z1b1")
            nc.vector.tensor_copy(z1b1[:], zb[:])

            # outputs
            nc.sync.dma_start(z2_o[:], z2[:])
            nc.sync.dma_start(sums2_o[:], sums2[:])
            nc.sync.dma_start(z1b1_o[:], z1b1[:])
    nc.compile()
    return nc


def _build_p2():
    """Launch 2: affine2 -> a2 -> L3 -> z3,sums3 ; branch1 tail; branch2 full."""
    nc, bass, tile, mybir = _mk()
    dt = mybir.dt.float32
    Relu = mybir.ActivationFunctionType.Relu

    z2_i = nc.dram_tensor("z2_i", [100, ML], dt, kind="ExternalInput").ap()
    gs2 = nc.dram_tensor("gs2", [100, 2], dt, kind="ExternalInput").ap()
    z1g = nc.dram_tensor("z1g", [100, 16], dt, kind="ExternalInput").ap()
    tT = nc.dram_tensor("tT", [1, 100], dt, kind="ExternalInput").ap()
    w3tT = nc.dram_tensor("w3tT", [100, 100], dt, kind="ExternalInput").ap()
    wb1 = nc.dram_tensor("wb1", [100, 300], dt, kind="ExternalInput").ap()  # w2,w3,w4 b1T
    b4b1 = nc.dram_tensor("b4b1", [100, 1], dt, kind="ExternalInput").ap()
    w1b2T = nc.dram_tensor("w1b2T", [1, 100], dt, kind="ExternalInput").ap()
    wb2 = nc.dram_tensor("wb2", [100, 300], dt, kind="ExternalInput").ap()
    b4b2 = nc.dram_tensor("b4b2", [100, 1], dt, kind="ExternalInput").ap()
    gbs = nc.dram_tensor("gbs", [100, 14], dt, kind="ExternalInput").ap()
    # gbs cols: 0:2 trunk-BN2, 2:8 b1-BN1..3, 8:14 b2-BN1..3

    z3_o = nc.dram_tensor("z3_o", [100, ML], dt, kind="ExternalOutput").ap()
    sums3_o = nc.dram_tensor("sums3_o", [100, 2], dt, kind="ExternalOutput").ap()
    b1T_o = nc.dram_tensor("b1T_o", [100, 16], dt, kind="ExternalOutput").ap()
    b2T_o = nc.dram_tensor("b2T_o", [100, 100], dt, kind="ExternalOutput").ap()

    with tile.TileContext(nc) as tc:
        from contextlib import ExitStack
        with ExitStack() as ctx:
            sb = ctx.enter_context(tc.tile_pool(name="sb", bufs=1))
            sm = ctx.enter_context(tc.tile_pool(name="sm", bufs=1))
            ps = ctx.enter_context(tc.tile_pool(name="ps", bufs=2, space="PSUM"))
            psb = ctx.enter_context(tc.tile_pool(name="psb", bufs=2, space="PSUM"))

            z2_sb = sb.tile([100, ML], dt, tag="z2")
            nc.sync.dma_start(z2_sb[:], z2_i[:])
            gs2_sb = sm.tile([100, 2], dt, tag="gs2")
            nc.sync.dma_start(gs2_sb[:], gs2[:])
            z1g_sb = sm.tile([100, 16], dt, tag="z1g")
            nc.sync.dma_start(z1g_sb[:], z1g[:])
            tT_sb = sm.tile([1, 100], dt, tag="tT")
            nc.sync.dma_start(tT_sb[:], tT[:])
            w3t_sb = sb.tile([100, 100], dt, tag="w3t")
            nc.sync.dma_start(w3t_sb[:], w3tT[:])
            wb1_sb = sb.tile([100, 300], dt, tag="wb1")
            nc.sync.dma_start(wb1_sb[:], wb1[:])
            b4b1_sb = sm.tile([100, 1], dt, tag="b4b1")
            nc.sync.dma_start(b4b1_sb[:], b4b1[:])
            w1b2_sb = sm.tile([1, 100], dt, tag="w1b2")
            nc.sync.dma_start(w1b2_sb[:], w1b2T[:])
            wb2_sb = sb.tile([100, 300], dt, tag="wb2")
            nc.sync.dma_start(wb2_sb[:], wb2[:])
            b4b2_sb = sm.tile([100, 1], dt, tag="b4b2")
            nc.sync.dma_start(b4b2_sb[:], b4b2[:])
            gbs_sb = sm.tile([100, 14], dt, tag="gbs")
            nc.sync.dma_start(gbs_sb[:], gbs[:])

            # trunk: affine2 -> a2 -> L3 -> z3, stats
            s2, c2 = _bn_finalize(nc, mybir, sm, gs2_sb, gbs_sb[:, 0:2], M, "t2")
            a2 = sb.tile([100, ML], dt, tag="a2")
            nc.scalar.activation(a2[:], z2_sb[:], Relu, bias=c2[:], scale=s2[:])
            z3 = sb.tile([100, ML], dt, tag="z3")
            st6 = sm.tile([100, 4, 6], dt, tag="st6z3")
            for ci in range(4):
                zp = ps.tile([100, 512], dt, tag="zp")
                nc.tensor.matmul(zp[:], w3t_sb[:], a2[:, ci * 512:(ci + 1) * 512],
                                 start=True, stop=True)
                nc.vector.bn_stats(st6[:, ci, :], zp[:])
                nc.scalar.copy(z3[:, ci * 512:(ci + 1) * 512], zp[:])
            mv = sm.tile([100, 2], dt, tag="mvz3")
            nc.vector.bn_aggr(mv[:], st6[:])
            sums3 = sm.tile([100, 2], dt, tag="sums3")
            nc.vector.tensor_mul(sums3[:, 1:2], mv[:, 0:1], mv[:, 0:1])
            nc.vector.tensor_add(sums3[:, 1:2], sums3[:, 1:2], mv[:, 1:2])
            nc.vector.tensor_scalar_mul(sums3[:, 0:1], mv[:, 0:1], float(ML))
            nc.vector.tensor_scalar_mul(sums3[:, 1:2], sums3[:, 1:2], float(ML))

            # branch1 tail
            wsb1 = [wb1_sb[:, 0:100], wb1_sb[:, 100:200], wb1_sb[:, 200:300]]
            b1T = _mlp_tail(nc, mybir, sm, psb, z1g_sb, 16, wsb1, b4b1_sb,
                            gbs_sb[:, 2:8], "b1")
            # branch2 full
            zb2 = psb.tile([100, 100], dt, tag="zb2")
            nc.tensor.matmul(zb2[:], w1b2_sb[:], tT_sb[:], start=True, stop=True)
            z1b2 = sm.tile([100, 100], dt, tag="z1b2")
            nc.vector.tensor_copy(z1b2[:], zb2[:])
            wsb2 = [wb2_sb[:, 0:100], wb2_sb[:, 100:200], wb2_sb[:, 200:300]]
            b2T = _mlp_tail(nc, mybir, sm, psb, z1b2, 100, wsb2, b4b2_sb,
                            gbs_sb[:, 8:14], "b2")

            nc.sync.dma_start(z3_o[:], z3[:])
            nc.sync.dma_start(sums3_o[:], sums3[:])
            nc.sync.dma_start(b1T_o[:], b1T[:])
            nc.sync.dma_start(b2T_o[:], b2T[:])
    nc.compile()
    return nc


def _build_p3():
    """Launch 3: affine3 -> a3aug ; A'aug = [W4|b4]^T AT ; contraction; out."""
    nc, bass, tile, mybir = _mk()
    dt = mybir.dt.float32
    Relu = mybir.ActivationFunctionType.Relu

    z3_i = nc.dram_tensor("z3_i", [100, ML], dt, kind="ExternalInput").ap()
    gs3 = nc.dram_tensor("gs3", [100, 2], dt, kind="ExternalInput").ap()
    b1T_i = nc.dram_tensor("b1T_i", [100, 16], dt, kind="ExternalInput").ap()
    b2T_i = nc.dram_tensor("b2T_i", [100, 100], dt, kind="ExternalInput").ap()
    w4tT = nc.dram_tensor("w4tT", [100, 100], dt, kind="ExternalInput").ap()
    b4t = nc.dram_tensor("b4t", [100, 1], dt, kind="ExternalInput").ap()
    gb3 = nc.dram_tensor("gb3", [100, 2], dt, kind="ExternalInput").ap()
    out = nc.dram_tensor("out", [NB, ML, T], dt, kind="ExternalOutput").ap()

    with tile.TileContext(nc) as tc:
        from contextlib import ExitStack
        with ExitStack() as ctx:
            sb = ctx.enter_context(tc.tile_pool(name="sb", bufs=1))
            sm = ctx.enter_context(tc.tile_pool(name="sm", bufs=1))
            osb = ctx.enter_context(tc.tile_pool(name="osb", bufs=3))
            ps = ctx.enter_context(tc.tile_pool(name="ps", bufs=2, space="PSUM"))
            psc = ctx.enter_context(tc.tile_pool(name="psc", bufs=2, space="PSUM"))

            z3_sb = sb.tile([100, ML], dt, tag="z3")
            nc.sync.dma_start(z3_sb[:], z3_i[:])
            gs3_sb = sm.tile([100, 2], dt, tag="gs3")
            nc.sync.dma_start(gs3_sb[:], gs3[:])
            b1T_sb = sm.tile([100, 16], dt, tag="b1T")
            nc.sync.dma_start(b1T_sb[:], b1T_i[:])
            b2T_sb = sm.tile([100, 100], dt, tag="b2T")
            nc.sync.dma_start(b2T_sb[:], b2T_i[:])
            w4_sb = sb.tile([100, 100], dt, tag="w4")
            nc.sync.dma_start(w4_sb[:], w4tT[:])
            b4_sb = sm.tile([100, 1], dt, tag="b4")
            nc.sync.dma_start(b4_sb[:], b4t[:])
            gb3_sb = sm.tile([100, 2], dt, tag="gb3")
            nc.sync.dma_start(gb3_sb[:], gb3[:])

            Ident = mybir.ActivationFunctionType.Identity
            # affine3 -> a3 ; L4 (+b4) -> trT
            s3, c3 = _bn_finalize(nc, mybir, sm, gs3_sb, gb3_sb, M, "t3")
            a3 = sb.tile([100, ML], dt, tag="a3")
            nc.scalar.activation(a3[:], z3_sb[:], Relu, bias=c3[:], scale=s3[:])
            trT = sb.tile([100, ML], dt, tag="trT")
            for ci in range(4):
                zp = ps.tile([100, 512], dt, tag="zp")
                nc.tensor.matmul(zp[:], w4_sb[:], a3[:, ci * 512:(ci + 1) * 512],
                                 start=True, stop=True)
                nc.scalar.activation(trT[:, ci * 512:(ci + 1) * 512], zp[:],
                                     Ident, bias=b4_sb[:], scale=1.0)

            # AT [100, 1600]: AT[:, n*100:(n+1)*100] = b2T * b1T[:, n]
            AT = sb.tile([100, 1600], dt, tag="AT")
            for n in range(NB):
                nc.vector.tensor_scalar_mul(AT[:, n * 100:(n + 1) * 100],
                                            b2T_sb[:], b1T_sb[:, n:n + 1])

            # contraction: per m-tile (16 of 128), two halves of 800 cols
            for mt in range(16):
                ot = osb.tile([128, 1600], dt, tag="ot")
                for hf in range(2):
                    cp = psc.tile([128, 1024], dt, tag="cp")
                    base = hf * 800
                    nc.tensor.matmul(cp[:, 0:512],
                                     trT[:, mt * 128:(mt + 1) * 128],
                                     AT[:, base:base + 512],
                                     start=True, stop=True)
                    nc.tensor.matmul(cp[:, 512:800],
                                     trT[:, mt * 128:(mt + 1) * 128],
                                     AT[:, base + 512:base + 800],
                                     start=True, stop=True)
                    nc.scalar.copy(ot[:, base:base + 400], cp[:, 0:400])
                    nc.vector.tensor_copy(ot[:, base + 400:base + 800],
                                          cp[:, 400:800])
                # one DMA per m-tile: src [128,16,100] -> out[n, m-tile, t]
                dst = _ap(bass, out, mt * 128 * T,
                          [[T, 128], [ML * T, NB], [1, T]])
                src = ot[:, :].rearrange("p (n t) -> p n t", n=NB)
                nc.sync.dma_start(dst, src)
    nc.compile()
    return nc


def _get_programs():
    if "p1" not in _CACHE:
        _CACHE["p1"] = _build_p1()
        _CACHE["p2"] = _build_p2()
        _CACHE["p3"] = _build_p3()
    return _CACHE["p1"], _CACHE["p2"], _CACHE["p3"]


def _run(nc, in_maps, **kw):
    from concourse import bass_utils
    return bass_utils.run_bass_kernel_spmd(nc, in_maps,
                                           core_ids=list(range(N_CORES)), **kw)


def kernel(x, points, times, branch1_params, branch2_params, trunk_params,
           _timings=None):
    x = _f32(x); points = _f32(points); times = _f32(times)
    b1p = [tuple(_f32(t) for t in tup) for tup in branch1_params]
    b2p = [tuple(_f32(t) for t in tup) for tup in branch2_params]
    trp = [tuple(_f32(t) for t in tup) for tup in trunk_params]
    # params: [(W1,b1),(g1,be1),(W2,b2),(g2,be2),(W3,b3),(g3,be3),(W4,b4)]
    p1, p2, p3 = _get_programs()

    pT_full = np.ascontiguousarray(points.T)              # [2, 16384]
    pfold = np.ascontiguousarray(points.reshape(128, 256))
    xT_full = np.ascontiguousarray(x.T)                   # [16384, 16]
    w1b1T_full = np.ascontiguousarray(b1p[0][0].T)        # [16384, 100]

    def gb(p, i):  # (gamma, beta) of BN i (0,1,2) as [100,2]
        g, be = p[2 * i + 1]
        return np.ascontiguousarray(np.stack([g, be], axis=1))

    in1 = []
    for c in range(N_CORES):
        in1.append({
            "pT": np.ascontiguousarray(pT_full[:, c * ML:(c + 1) * ML]),
            "pfold": pfold,
            "xT": np.ascontiguousarray(xT_full[c * KSH:(c + 1) * KSH]),
            "w1b1T": np.ascontiguousarray(w1b1T_full[c * KSH:(c + 1) * KSH]),
            "w1t_raw": trp[0][0],                          # [100,2]
            "w1tT": np.ascontiguousarray(trp[0][0].T),     # [2,100]
            "w2tT": np.ascontiguousarray(trp[2][0].T),     # [100,100]
            "gb1": gb(trp, 0),
        })
    r1 = _run(p1, in1, trace=bool(_timings))
    if _timings is not None:
        _timings.append(r1.exec_time_ns)

    z1g = np.sum([r1.results[c]["z1b1_o"].astype(np.float64)
                  for c in range(N_CORES)], axis=0).astype(F32)
    gs2 = np.sum([r1.results[c]["sums2_o"].astype(np.float64)
                  for c in range(N_CORES)], axis=0).astype(F32)

    def wpack(p):
        return np.ascontiguousarray(np.concatenate(
            [p[2][0].T, p[4][0].T, p[6][0].T], axis=1))    # [100, 300]

    gbs = np.concatenate([gb(trp, 1), gb(b1p, 0), gb(b1p, 1), gb(b1p, 2),
                          gb(b2p, 0), gb(b2p, 1), gb(b2p, 2)], axis=1)
    in2 = []
    for c in range(N_CORES):
        in2.append({
            "z2_i": r1.results[c]["z2_o"],
            "gs2": gs2, "z1g": z1g,
            "tT": np.ascontiguousarray(times.T),           # [1,100]
            "w3tT": np.ascontiguousarray(trp[4][0].T),
            "wb1": wpack(b1p), "b4b1": b1p[6][1].reshape(100, 1),
            "w1b2T": np.ascontiguousarray(b2p[0][0].T),    # [1,100]
            "wb2": wpack(b2p), "b4b2": b2p[6][1].reshape(100, 1),
            "gbs": np.ascontiguousarray(gbs),
        })
    r2 = _run(p2, in2, trace=bool(_timings))
    if _timings is not None:
        _timings.append(r2.exec_time_ns)

    gs3 = np.sum([r2.results[c]["sums3_o"].astype(np.float64)
                  for c in range(N_CORES)], axis=0).astype(F32)
    in3 = []
    for c in range(N_CORES):
        in3.append({
            "z3_i": r2.results[c]["z3_o"],
            "gs3": gs3,
            "b1T_i": r2.results[c]["b1T_o"],
            "b2T_i": r2.results[c]["b2T_o"],
            "w4tT": np.ascontiguousarray(trp[6][0].T),
            "b4t": trp[6][1].reshape(100, 1),
            "gb3": gb(trp, 2),
        })
    r3 = _run(p3, in3, trace=bool(_timings))
    if _timings is not None:
        _timings.append(r3.exec_time_ns)

    return np.concatenate([r3.results[c]["out"] for c in range(N_CORES)],
                          axis=1)
